# revision 15
# baseline (speedup 1.0000x reference)
"""Trainium2 Bass kernel for nn_Interpolator: pilot-to-subcarrier linear
interpolation with learned per-subcarrier weights.

Math: out[b, t] = alpha[t] * Hp[b, right[t]] + beta[t] * Hp[b, left[t]]
where Hp = [H, extrapolated last column] and left/right come from a
searchsorted of subcarrier indices against (0-based) pilot positions.

The op is linear in H, so it collapses to out = H @ W with a sparse
W [256, 4096] built on the host from (pilot_loc, alpha, beta); the
extrapolation column folds into W's last two rows.

On-device this is a TensorE matmul in bf16. The rel-err budget (2e-2)
is far above bf16 rounding (~1e-3), so H is sent as plain bf16 (no
error-compensation terms) and the output is stored as fp16 — the
kernel is DMA-bound and fp16 halves the dominant store traffic. If W
is not exactly bf16-representable, a compensating hi@W_lo term is
added. Per 512-wide output chunk only the 128-row halves of W that
are nonzero are contracted (full-K slices keep every matmul at PE
tile_position (0,0) — mixing sub-128 tile_positions across
accumulation groups crashes the device).

Layout choices, all serving the DMA/drain pipeline:
- H arrives pre-transposed from the host as hT [2*P, BS] bf16
  (real rows then imag rows), so the PE does no transposes and the
  DVE does no transpose drains; matmul lhsT (stationary) slices are
  direct SBUF views.
- PSUM tiles are [128, 2, 512] f32: the real matmul group fills
  [:, 0, :], imag fills [:, 1, :], and ONE cast per chunk drains both
  to fp16 (PSUM reads run the DVE at 1x regardless of dtype, so fewer
  bigger drains win). Drains alternate DVE/ACT 1:1.
- DRAM out is [BS, 8192] fp16, real block then imag block; the drain's
  3D dst AP writes both blocks in one instruction. Host interleaves
  r/i and upcasts to f32 while unsharding.

Sharding: data-parallel over the batch dim, 2048 rows per core x 8 cores.
"""

import os
import sys

if os.path.isdir("/opt/trn_rl_repo") and "/opt/trn_rl_repo" not in sys.path:
    sys.path.insert(0, "/opt/trn_rl_repo")

import ml_dtypes
import numpy as np

_BF16 = np.dtype(ml_dtypes.bfloat16)

_B, _P, _NFFT = 16384, 256, 4096
_NC = 8
_BS = _B // _NC          # rows per core
_PT = 128                # partition tile (batch rows per tile)
_NBT = _BS // _PT        # batch tiles per core
_CH = 512                # output-chunk width (one PSUM bank of fp32)
_NCHUNK = _NFFT // _CH

_cache = {}


def _interp_matrix(pilot_loc, alpha, beta):
    """W [256, 4096] f32 such that out = H @ W reproduces the reference."""
    p = pilot_loc.astype(np.float64) - 1.0  # reference: 1-based -> 0-based
    pp = np.concatenate([p, [float(_NFFT - 1)]])
    t = np.arange(_NFFT)
    left = np.clip(np.searchsorted(pp, t, side="right") - 1, 0, _P - 1)
    right = left + 1
    Wf = np.zeros((_P + 1, _NFFT), np.float64)
    Wf[left, t] += beta.astype(np.float64)
    Wf[right, t] += alpha.astype(np.float64)
    # Hp[:, P] = H[:, P-1] + slope * (NFFT-1 - p[-1]),
    # slope = (H[:, P-1] - H[:, P-2]) / (p[-1] - p[-2])  -> linear in H.
    d = (float(_NFFT - 1) - p[-1]) / (p[-1] - p[-2])
    W = Wf[:_P]
    W[_P - 1] += (1.0 + d) * Wf[_P]
    W[_P - 2] += (-d) * Wf[_P]
    return np.ascontiguousarray(W.astype(np.float32))


def _chunk_pieces(W):
    """Per 512-col chunk: which 128-row halves of W have any nonzeros."""
    out = []
    for c in range(_NCHUNK):
        cols = W[:, c * _CH:(c + 1) * _CH]
        nz = np.nonzero(np.any(cols != 0.0, axis=1))[0]
        k_lo, k_hi = int(nz.min()), int(nz.max())
        pieces = []
        for half in (0, 1):
            if k_lo <= 128 * half + 127 and k_hi >= 128 * half:
                pieces.append(half)
        out.append(tuple(pieces))
    return tuple(out)


def _bf16_split(x):
    hi = x.astype(_BF16)
    lo = (x - hi.astype(np.float32)).astype(_BF16)
    return hi, lo


def _build_program(pieces_per_chunk, use_wlo, store_every=4,
                   copy_cycle="vs", store_rings="sa"):
    from contextlib import ExitStack

    import concourse.bacc as bacc
    import concourse.bass as bass
    import concourse.mybir as mybir
    import concourse.tile as tile

    f32 = mybir.dt.float32
    f16 = mybir.dt.float16
    bf16 = mybir.dt.bfloat16

    nc = bacc.Bacc("TRN2", target_bir_lowering=False, debug=False,
                   num_devices=_NC)
    # Pre-transposed input: rows [hr^T (256) | hi^T (256)], cols = batch.
    ht_in = nc.dram_tensor("ht", [4 * 128, _BS], bf16,
                           kind="ExternalInput").ap()
    w_in = {"h": nc.dram_tensor("wh", [_P, _NFFT], bf16,
                                kind="ExternalInput").ap()}
    if use_wlo:
        w_in["l"] = nc.dram_tensor("wl", [_P, _NFFT], bf16,
                                   kind="ExternalInput").ap()
    # real block then imag block; host interleaves + upcasts.
    out = nc.dram_tensor("out", [_BS, 2 * _NFFT], f16,
                         kind="ExternalOutput").ap()

    with tile.TileContext(nc) as tc, ExitStack() as ctx:
        const_pool = ctx.enter_context(tc.tile_pool(name="const", bufs=1))
        out_pool = ctx.enter_context(tc.tile_pool(name="outp", bufs=3))
        ps_mm = ctx.enter_context(tc.tile_pool(name="psm", bufs=4,
                                               space="PSUM"))

        # hT SBUF tiles: (x, half) -> [128, BS]; the load order below is
        # chosen so the first chunks' operands land first: the sync ring
        # is idle until the first store, so it carries the h0 hT tiles
        # while the scalar ring streams W (in column halves) and the h1
        # tiles.
        hT = {}
        for x in ("r", "i"):
            for h in (0, 1):
                hT[(x, h)] = const_pool.tile([128, _BS], bf16,
                                             tag=f"hT{x}{h}",
                                             name=f"hT{x}{h}")
        w_sb = {}
        for part in w_in:
            for h in (0, 1):
                w_sb[(part, h)] = const_pool.tile([128, _NFFT], bf16,
                                                  tag=f"w{part}{h}",
                                                  name=f"w{part}{h}")
        nc.sync.dma_start(hT[("r", 0)][:], ht_in[0:128, :])
        nc.sync.dma_start(hT[("i", 0)][:], ht_in[256:384, :])
        half_w = _NFFT // 2
        nc.scalar.dma_start(w_sb[("h", 0)][:, 0:half_w],
                            w_in["h"][0:128, 0:half_w])
        nc.scalar.dma_start(w_sb[("h", 0)][:, half_w:],
                            w_in["h"][0:128, half_w:])
        nc.scalar.dma_start(hT[("r", 1)][:], ht_in[128:256, :])
        nc.scalar.dma_start(hT[("i", 1)][:], ht_in[384:512, :])
        nc.scalar.dma_start(w_sb[("h", 1)][:], w_in["h"][128:256, :])
        if use_wlo:
            for h in (0, 1):
                nc.scalar.dma_start(
                    w_sb[("l", h)][:], w_in["l"][128 * h:128 * (h + 1), :])

        terms = [("h", "h")]
        if use_wlo:
            terms.append(("h", "l"))

        ring_of = {"s": nc.sync, "a": nc.scalar, "g": nc.gpsimd}
        copy_idx = 0
        store_idx = 0
        for bt in range(_NBT):
            # first tile: fine-grained stores so the write ring starts
            # as early as possible; last tile: fine-grained to shrink
            # the tail drain after the final matmul.
            se = 2 if bt in (0, _NBT - 1) else store_every
            bsl = slice(128 * bt, 128 * (bt + 1))
            ot = out_pool.tile([128, 2, _NFFT], f16, tag="ot")
            for c in range(_NCHUNK):
                pieces = pieces_per_chunk[c]
                n_mm = len(pieces) * len(terms)
                ps = ps_mm.tile([128, 2, _CH], f32, tag="ps")
                for xi, x in enumerate(("r", "i")):
                    j = 0
                    for h in pieces:
                        for (hp, wp) in terms:
                            nc.tensor.matmul(
                                ps[:, xi, :],
                                hT[(x, h)][:, bsl],
                                w_sb[(wp, h)][:, c * _CH:(c + 1) * _CH],
                                start=(j == 0),
                                stop=(j == n_mm - 1),
                            )
                            j += 1
                # one drain for the r+i pair; PSUM reads run DVE/ACT at
                # 1x, so fewer bigger casts win. Alternate engines 1:1.
                dst = ot[:, :, _CH * c:_CH * (c + 1)]
                eng = copy_cycle[copy_idx % len(copy_cycle)]
                if eng == "s":
                    nc.scalar.copy(dst, ps[:])
                else:
                    nc.vector.tensor_copy(dst, ps[:])
                copy_idx += 1
                if se >= 9:
                    # whole-tile single store after the last chunk:
                    # r and i blocks are adjacent in both SBUF and DRAM,
                    # so one DMA with 16KB/row descriptors covers both.
                    if c == _NCHUNK - 1:
                        ring = ring_of[store_rings[store_idx
                                                   % len(store_rings)]]
                        ring.dma_start(out[bass.ts(bt, 128), :],
                                       ot[:, :, :])
                        store_idx += 1
                elif (c + 1) % se == 0:
                    # store finished slices early; keeps the write ring
                    # fed and shrinks the tail drain. Alternating rings
                    # overlaps the per-ring DGE trigger latency between
                    # consecutive stores.
                    w0 = _CH * (c + 1 - se)
                    w1 = _CH * (c + 1)
                    for parity in (0, 1):
                        ring = ring_of[store_rings[store_idx
                                                   % len(store_rings)]]
                        ring.dma_start(
                            out[bass.ts(bt, 128),
                                _NFFT * parity + w0:_NFFT * parity + w1],
                            ot[:, parity, w0:w1])
                        store_idx += 1

    nc.compile()
    return nc


def _get_program(pieces, use_wlo):
    # experiment knobs (default values are the tuned ones)
    se = int(os.environ.get("K_STORE_EVERY", "4"))
    cc = os.environ.get("K_COPY_CYCLE", "vs")
    sr = os.environ.get("K_STORE_RINGS", "sa")
    key = (pieces, use_wlo, se, cc, sr)
    prog = _cache.get(key)
    if prog is None:
        prog = _build_program(pieces, use_wlo, store_every=se,
                              copy_cycle=cc, store_rings=sr)
        _cache[key] = prog
    return prog


def _make_in_maps(H_real, H_imag, W):
    w_hi, w_lo = _bf16_split(W)
    use_wlo = bool(np.any(np.asarray(w_lo) != 0))
    in_maps = []
    for i in range(_NC):
        sl = slice(i * _BS, (i + 1) * _BS)
        ht = np.ascontiguousarray(np.concatenate(
            [H_real[sl].astype(_BF16).T, H_imag[sl].astype(_BF16).T],
            axis=0))
        m = {"ht": ht, "wh": w_hi}
        if use_wlo:
            m["wl"] = w_lo
        in_maps.append(m)
    return in_maps, use_wlo


def kernel(H_real, H_imag, pilot_loc, alpha, beta):
    H_real = np.ascontiguousarray(np.asarray(H_real, dtype=np.float32))
    H_imag = np.ascontiguousarray(np.asarray(H_imag, dtype=np.float32))
    pilot_loc = np.asarray(pilot_loc, dtype=np.float32)
    alpha = np.asarray(alpha, dtype=np.float32)
    beta = np.asarray(beta, dtype=np.float32)

    W = _interp_matrix(pilot_loc, alpha, beta)
    in_maps, use_wlo = _make_in_maps(H_real, H_imag, W)
    nc = _get_program(_chunk_pieces(W), use_wlo)

    from concourse.bass_utils import run_bass_kernel_spmd

    res = run_bass_kernel_spmd(nc, in_maps, list(range(_NC))).results
    full = np.empty((_B, _NFFT, 2), dtype=np.float32)
    for i, r in enumerate(res):
        o = r["out"]
        full[i * _BS:(i + 1) * _BS, :, 0] = o[:, :_NFFT]
        full[i * _BS:(i + 1) * _BS, :, 1] = o[:, _NFFT:]
    return full


# revision 22
# speedup vs baseline: 1.0529x; 1.0529x over previous
"""Trainium2 Bass kernel for nn_Interpolator: pilot-to-subcarrier linear
interpolation with learned per-subcarrier weights.

Math: out[b, t] = alpha[t] * Hp[b, right[t]] + beta[t] * Hp[b, left[t]]
where Hp = [H, extrapolated last column] and left/right come from a
searchsorted of subcarrier indices against (0-based) pilot positions.

The op is linear in H, so it collapses to out = H @ W with a sparse
W [256, 4096] built on the host from (pilot_loc, alpha, beta); the
extrapolation column folds into W's last two rows.

On-device this is a TensorE matmul in bf16. The rel-err budget (2e-2)
is far above bf16 rounding (~1e-3), so H is sent as plain bf16 (no
error-compensation terms) and the output is stored as fp16 — the
kernel is DMA-bound and fp16 halves the dominant store traffic. If W
is not exactly bf16-representable, a compensating hi@W_lo term is
added. Per 512-wide output chunk only the 128-row halves of W that
are nonzero are contracted (full-K slices keep every matmul at PE
tile_position (0,0) — mixing sub-128 tile_positions across
accumulation groups crashes the device).

Layout choices, all serving the DMA/drain pipeline:
- H arrives pre-transposed from the host as hT [2*P, BS] bf16
  (real rows then imag rows), so the PE does no transposes and the
  DVE does no transpose drains; matmul lhsT (stationary) slices are
  direct SBUF views.
- PSUM tiles are [128, 2, 512] f32: the real matmul group fills
  [:, 0, :], imag fills [:, 1, :], and ONE cast per chunk drains both
  to fp16 (PSUM reads run the DVE at 1x regardless of dtype, so fewer
  bigger drains win). Drains alternate DVE/ACT 1:1.
- DRAM out is [BS, 8192] fp16, real block then imag block; the drain's
  3D dst AP writes both blocks in one instruction. Host interleaves
  r/i and upcasts to f32 while unsharding.

Sharding: data-parallel over the batch dim, 2048 rows per core x 8 cores.
"""

import os
import sys

if os.path.isdir("/opt/trn_rl_repo") and "/opt/trn_rl_repo" not in sys.path:
    sys.path.insert(0, "/opt/trn_rl_repo")

import ml_dtypes
import numpy as np

_BF16 = np.dtype(ml_dtypes.bfloat16)

_B, _P, _NFFT = 16384, 256, 4096
_NC = 8
_BS = _B // _NC          # rows per core
_PT = 128                # partition tile (batch rows per tile)
_NBT = _BS // _PT        # batch tiles per core
_CH = 512                # output-chunk width (one PSUM bank of fp32)
_NCHUNK = _NFFT // _CH

_cache = {}


def _interp_matrix(pilot_loc, alpha, beta):
    """W [256, 4096] f32 such that out = H @ W reproduces the reference."""
    p = pilot_loc.astype(np.float64) - 1.0  # reference: 1-based -> 0-based
    pp = np.concatenate([p, [float(_NFFT - 1)]])
    t = np.arange(_NFFT)
    left = np.clip(np.searchsorted(pp, t, side="right") - 1, 0, _P - 1)
    right = left + 1
    Wf = np.zeros((_P + 1, _NFFT), np.float64)
    Wf[left, t] += beta.astype(np.float64)
    Wf[right, t] += alpha.astype(np.float64)
    # Hp[:, P] = H[:, P-1] + slope * (NFFT-1 - p[-1]),
    # slope = (H[:, P-1] - H[:, P-2]) / (p[-1] - p[-2])  -> linear in H.
    d = (float(_NFFT - 1) - p[-1]) / (p[-1] - p[-2])
    W = Wf[:_P]
    W[_P - 1] += (1.0 + d) * Wf[_P]
    W[_P - 2] += (-d) * Wf[_P]
    return np.ascontiguousarray(W.astype(np.float32))


def _chunk_pieces(W):
    """Per 512-col chunk: which 128-row halves of W have any nonzeros."""
    out = []
    for c in range(_NCHUNK):
        cols = W[:, c * _CH:(c + 1) * _CH]
        nz = np.nonzero(np.any(cols != 0.0, axis=1))[0]
        k_lo, k_hi = int(nz.min()), int(nz.max())
        pieces = []
        for half in (0, 1):
            if k_lo <= 128 * half + 127 and k_hi >= 128 * half:
                pieces.append(half)
        out.append(tuple(pieces))
    return tuple(out)


def _bf16_split(x):
    hi = x.astype(_BF16)
    lo = (x - hi.astype(np.float32)).astype(_BF16)
    return hi, lo


def _build_program(pieces_per_chunk, use_wlo, store_every=9,
                   copy_cycle="vs", store_rings="s", edge_se=2,
                   edge_tiles=(0, _NBT - 1)):
    from contextlib import ExitStack

    import concourse.bacc as bacc
    import concourse.bass as bass
    import concourse.mybir as mybir
    import concourse.tile as tile

    f32 = mybir.dt.float32
    f16 = mybir.dt.float16
    bf16 = mybir.dt.bfloat16

    nc = bacc.Bacc("TRN2", target_bir_lowering=False, debug=False,
                   num_devices=_NC)
    # Pre-transposed input: rows [hr^T (256) | hi^T (256)], cols = batch.
    ht_in = nc.dram_tensor("ht", [4 * 128, _BS], bf16,
                           kind="ExternalInput").ap()
    w_in = {"h": nc.dram_tensor("wh", [_P, _NFFT], bf16,
                                kind="ExternalInput").ap()}
    if use_wlo:
        w_in["l"] = nc.dram_tensor("wl", [_P, _NFFT], bf16,
                                   kind="ExternalInput").ap()
    # real block then imag block; host interleaves + upcasts.
    out = nc.dram_tensor("out", [_BS, 2 * _NFFT], f16,
                         kind="ExternalOutput").ap()

    with tile.TileContext(nc) as tc, ExitStack() as ctx:
        const_pool = ctx.enter_context(tc.tile_pool(name="const", bufs=1))
        out_pool = ctx.enter_context(tc.tile_pool(name="outp", bufs=3))
        ps_mm = ctx.enter_context(tc.tile_pool(name="psm", bufs=4,
                                               space="PSUM"))

        # hT SBUF tiles: (x, half) -> [128, BS]; the load order below is
        # chosen so the first chunks' operands land first: the sync ring
        # is idle until the first store, so it carries the h0 hT tiles
        # while the scalar ring streams W (in column halves) and the h1
        # tiles.
        hT = {}
        for x in ("r", "i"):
            for h in (0, 1):
                hT[(x, h)] = const_pool.tile([128, _BS], bf16,
                                             tag=f"hT{x}{h}",
                                             name=f"hT{x}{h}")
        w_sb = {}
        for part in w_in:
            for h in (0, 1):
                w_sb[(part, h)] = const_pool.tile([128, _NFFT], bf16,
                                                  tag=f"w{part}{h}",
                                                  name=f"w{part}{h}")
        nc.sync.dma_start(hT[("r", 0)][:], ht_in[0:128, :])
        nc.sync.dma_start(hT[("i", 0)][:], ht_in[256:384, :])
        half_w = _NFFT // 2
        nc.scalar.dma_start(w_sb[("h", 0)][:, 0:half_w],
                            w_in["h"][0:128, 0:half_w])
        nc.scalar.dma_start(w_sb[("h", 0)][:, half_w:],
                            w_in["h"][0:128, half_w:])
        nc.scalar.dma_start(hT[("r", 1)][:], ht_in[128:256, :])
        nc.scalar.dma_start(hT[("i", 1)][:], ht_in[384:512, :])
        nc.scalar.dma_start(w_sb[("h", 1)][:], w_in["h"][128:256, :])
        if use_wlo:
            for h in (0, 1):
                nc.scalar.dma_start(
                    w_sb[("l", h)][:], w_in["l"][128 * h:128 * (h + 1), :])

        terms = [("h", "h")]
        if use_wlo:
            terms.append(("h", "l"))

        ring_of = {"s": nc.sync, "a": nc.scalar, "g": nc.gpsimd}
        copy_idx = 0
        store_idx = 0
        for bt in range(_NBT):
            # first tile(s): fine-grained stores so the write ring starts
            # as early as possible while the pipeline (and PE p-state)
            # ramps; last tile: fine-grained to shrink the tail drain
            # after the final matmul.
            se = edge_se if bt in edge_tiles else store_every
            bsl = slice(128 * bt, 128 * (bt + 1))
            ot = out_pool.tile([128, 2, _NFFT], f16, tag="ot")
            for c in range(_NCHUNK):
                pieces = pieces_per_chunk[c]
                n_mm = len(pieces) * len(terms)
                ps = ps_mm.tile([128, 2, _CH], f32, tag="ps")
                for xi, x in enumerate(("r", "i")):
                    j = 0
                    for h in pieces:
                        for (hp, wp) in terms:
                            nc.tensor.matmul(
                                ps[:, xi, :],
                                hT[(x, h)][:, bsl],
                                w_sb[(wp, h)][:, c * _CH:(c + 1) * _CH],
                                start=(j == 0),
                                stop=(j == n_mm - 1),
                            )
                            j += 1
                # one drain for the r+i pair; PSUM reads run DVE/ACT at
                # 1x, so fewer bigger casts win. Alternate engines 1:1.
                dst = ot[:, :, _CH * c:_CH * (c + 1)]
                eng = copy_cycle[copy_idx % len(copy_cycle)]
                if eng == "s":
                    nc.scalar.copy(dst, ps[:])
                else:
                    nc.vector.tensor_copy(dst, ps[:])
                copy_idx += 1
                if se >= 9:
                    # whole-tile single store after the last chunk:
                    # r and i blocks are adjacent in both SBUF and DRAM,
                    # so one DMA with 16KB/row descriptors covers both.
                    if c == _NCHUNK - 1:
                        ring = ring_of[store_rings[store_idx
                                                   % len(store_rings)]]
                        ring.dma_start(out[bass.ts(bt, 128), :],
                                       ot[:, :, :])
                        store_idx += 1
                elif (c + 1) % se == 0:
                    # store finished slices early; keeps the write ring
                    # fed and shrinks the tail drain. Alternating rings
                    # overlaps the per-ring DGE trigger latency between
                    # consecutive stores.
                    w0 = _CH * (c + 1 - se)
                    w1 = _CH * (c + 1)
                    for parity in (0, 1):
                        ring = ring_of[store_rings[store_idx
                                                   % len(store_rings)]]
                        ring.dma_start(
                            out[bass.ts(bt, 128),
                                _NFFT * parity + w0:_NFFT * parity + w1],
                            ot[:, parity, w0:w1])
                        store_idx += 1

    nc.compile()
    return nc


def _get_program(pieces, use_wlo):
    # experiment knobs (default values are the tuned ones)
    se = int(os.environ.get("K_STORE_EVERY", "9"))
    cc = os.environ.get("K_COPY_CYCLE", "vs")
    sr = os.environ.get("K_STORE_RINGS", "s")
    ese = int(os.environ.get("K_EDGE_SE", "2"))
    et = tuple(int(t) for t in
               os.environ.get("K_EDGE_TILES", "0,15").split(","))
    key = (pieces, use_wlo, se, cc, sr, ese, et)
    prog = _cache.get(key)
    if prog is None:
        prog = _build_program(pieces, use_wlo, store_every=se,
                              copy_cycle=cc, store_rings=sr,
                              edge_se=ese, edge_tiles=et)
        _cache[key] = prog
    return prog


def _make_in_maps(H_real, H_imag, W):
    w_hi, w_lo = _bf16_split(W)
    use_wlo = bool(np.any(np.asarray(w_lo) != 0))
    in_maps = []
    for i in range(_NC):
        sl = slice(i * _BS, (i + 1) * _BS)
        ht = np.ascontiguousarray(np.concatenate(
            [H_real[sl].astype(_BF16).T, H_imag[sl].astype(_BF16).T],
            axis=0))
        m = {"ht": ht, "wh": w_hi}
        if use_wlo:
            m["wl"] = w_lo
        in_maps.append(m)
    return in_maps, use_wlo


def kernel(H_real, H_imag, pilot_loc, alpha, beta):
    H_real = np.ascontiguousarray(np.asarray(H_real, dtype=np.float32))
    H_imag = np.ascontiguousarray(np.asarray(H_imag, dtype=np.float32))
    pilot_loc = np.asarray(pilot_loc, dtype=np.float32)
    alpha = np.asarray(alpha, dtype=np.float32)
    beta = np.asarray(beta, dtype=np.float32)

    W = _interp_matrix(pilot_loc, alpha, beta)
    in_maps, use_wlo = _make_in_maps(H_real, H_imag, W)
    nc = _get_program(_chunk_pieces(W), use_wlo)

    from concourse.bass_utils import run_bass_kernel_spmd

    res = run_bass_kernel_spmd(nc, in_maps, list(range(_NC))).results
    full = np.empty((_B, _NFFT, 2), dtype=np.float32)
    for i, r in enumerate(res):
        o = r["out"]
        full[i * _BS:(i + 1) * _BS, :, 0] = o[:, :_NFFT]
        full[i * _BS:(i + 1) * _BS, :, 1] = o[:, _NFFT:]
    return full


# revision 23
# speedup vs baseline: 1.1355x; 1.0785x over previous
"""Trainium2 Bass kernel for nn_Interpolator: pilot-to-subcarrier linear
interpolation with learned per-subcarrier weights.

Math: out[b, t] = alpha[t] * Hp[b, right[t]] + beta[t] * Hp[b, left[t]]
where Hp = [H, extrapolated last column] and left/right come from a
searchsorted of subcarrier indices against (0-based) pilot positions.

The op is linear in H, so it collapses to out = H @ W with a sparse
W [256, 4096] built on the host from (pilot_loc, alpha, beta); the
extrapolation column folds into W's last two rows.

On-device this is a TensorE matmul in bf16. The rel-err budget (2e-2)
is far above bf16 rounding (~1e-3), so H is sent as plain bf16 (no
error-compensation terms) and the output is stored as fp16 — the
kernel is DMA-bound and fp16 halves the dominant store traffic. If W
is not exactly bf16-representable, a compensating hi@W_lo term is
added. Per 512-wide output chunk only the 128-row halves of W that
are nonzero are contracted (full-K slices keep every matmul at PE
tile_position (0,0) — mixing sub-128 tile_positions across
accumulation groups crashes the device).

Layout choices, all serving the DMA/drain pipeline:
- H arrives pre-transposed from the host as hT [2*P, BS] bf16
  (real rows then imag rows), so the PE does no transposes and the
  DVE does no transpose drains; matmul lhsT (stationary) slices are
  direct SBUF views.
- PSUM tiles are [128, 2, 512] f32: the real matmul group fills
  [:, 0, :], imag fills [:, 1, :], and ONE cast per chunk drains both
  to fp16 (PSUM reads run the DVE at 1x regardless of dtype, so fewer
  bigger drains win). Drains alternate DVE/ACT 1:1.
- DRAM out is [BS, 8192] fp16, real block then imag block; the drain's
  3D dst AP writes both blocks in one instruction. Host interleaves
  r/i and upcasts to f32 while unsharding.

Sharding: data-parallel over the batch dim, 2048 rows per core x 8 cores.
"""

import os
import sys

if os.path.isdir("/opt/trn_rl_repo") and "/opt/trn_rl_repo" not in sys.path:
    sys.path.insert(0, "/opt/trn_rl_repo")

import ml_dtypes
import numpy as np

_BF16 = np.dtype(ml_dtypes.bfloat16)

_B, _P, _NFFT = 16384, 256, 4096
_NC = 8
_BS = _B // _NC          # rows per core
_PT = 128                # partition tile (batch rows per tile)
_NBT = _BS // _PT        # batch tiles per core
_CH = 512                # output-chunk width (one PSUM bank of fp32)
_NCHUNK = _NFFT // _CH

_cache = {}


def _interp_matrix(pilot_loc, alpha, beta):
    """W [256, 4096] f32 such that out = H @ W reproduces the reference."""
    p = pilot_loc.astype(np.float64) - 1.0  # reference: 1-based -> 0-based
    pp = np.concatenate([p, [float(_NFFT - 1)]])
    t = np.arange(_NFFT)
    left = np.clip(np.searchsorted(pp, t, side="right") - 1, 0, _P - 1)
    right = left + 1
    Wf = np.zeros((_P + 1, _NFFT), np.float64)
    Wf[left, t] += beta.astype(np.float64)
    Wf[right, t] += alpha.astype(np.float64)
    # Hp[:, P] = H[:, P-1] + slope * (NFFT-1 - p[-1]),
    # slope = (H[:, P-1] - H[:, P-2]) / (p[-1] - p[-2])  -> linear in H.
    d = (float(_NFFT - 1) - p[-1]) / (p[-1] - p[-2])
    W = Wf[:_P]
    W[_P - 1] += (1.0 + d) * Wf[_P]
    W[_P - 2] += (-d) * Wf[_P]
    return np.ascontiguousarray(W.astype(np.float32))


def _chunk_pieces(W):
    """Per 512-col chunk: which 128-row halves of W have any nonzeros."""
    out = []
    for c in range(_NCHUNK):
        cols = W[:, c * _CH:(c + 1) * _CH]
        nz = np.nonzero(np.any(cols != 0.0, axis=1))[0]
        k_lo, k_hi = int(nz.min()), int(nz.max())
        pieces = []
        for half in (0, 1):
            if k_lo <= 128 * half + 127 and k_hi >= 128 * half:
                pieces.append(half)
        out.append(tuple(pieces))
    return tuple(out)


def _bf16_split(x):
    hi = x.astype(_BF16)
    lo = (x - hi.astype(np.float32)).astype(_BF16)
    return hi, lo


def _build_program(pieces_per_chunk, use_wlo, store_every=9,
                   copy_cycle="vs", store_rings="s", edge_se=2,
                   edge_tiles=(0, _NBT - 1)):
    from contextlib import ExitStack

    import concourse.bacc as bacc
    import concourse.bass as bass
    import concourse.mybir as mybir
    import concourse.tile as tile

    f32 = mybir.dt.float32
    f16 = mybir.dt.float16
    bf16 = mybir.dt.bfloat16

    nc = bacc.Bacc("TRN2", target_bir_lowering=False, debug=False,
                   num_devices=_NC)
    # Pre-transposed input: rows [hr^T (256) | hi^T (256)], cols = batch.
    ht_in = nc.dram_tensor("ht", [4 * 128, _BS], bf16,
                           kind="ExternalInput").ap()
    w_in = {"h": nc.dram_tensor("wh", [_P, _NFFT], bf16,
                                kind="ExternalInput").ap()}
    if use_wlo:
        w_in["l"] = nc.dram_tensor("wl", [_P, _NFFT], bf16,
                                   kind="ExternalInput").ap()
    # real block then imag block; host interleaves + upcasts.
    out = nc.dram_tensor("out", [_BS, 2 * _NFFT], f16,
                         kind="ExternalOutput").ap()

    with tile.TileContext(nc) as tc, ExitStack() as ctx:
        const_pool = ctx.enter_context(tc.tile_pool(name="const", bufs=1))
        out_pool = ctx.enter_context(tc.tile_pool(name="outp", bufs=3))
        ps_mm = ctx.enter_context(tc.tile_pool(name="psm", bufs=4,
                                               space="PSUM"))

        # hT SBUF tiles: (x, half) -> [128, BS]; the load order below is
        # chosen so the first chunks' operands land first: the sync ring
        # is idle until the first store, so it carries the h0 hT tiles
        # while the scalar ring streams W (in column halves) and the h1
        # tiles.
        hT = {}
        for x in ("r", "i"):
            for h in (0, 1):
                hT[(x, h)] = const_pool.tile([128, _BS], bf16,
                                             tag=f"hT{x}{h}",
                                             name=f"hT{x}{h}")
        w_sb = {}
        for part in w_in:
            for h in (0, 1):
                w_sb[(part, h)] = const_pool.tile([128, _NFFT], bf16,
                                                  tag=f"w{part}{h}",
                                                  name=f"w{part}{h}")
        # Starter slices first: the first 256 batch columns of each hT
        # part (tiles 0-1's lhsT) and the first 2048 W columns (chunks
        # 0-3), so tile-0 matmuls and stores begin ~2us earlier; the
        # bulk follows. hT h1 bulk stays OFF the sync ring — the store
        # queue is FIFO and a late 1MB load there would block tile-0's
        # stores behind it.
        bst = 2 * _PT  # starter width in batch columns
        half_w = _NFFT // 2
        nc.sync.dma_start(hT[("r", 0)][:, 0:bst], ht_in[0:128, 0:bst])
        nc.sync.dma_start(hT[("i", 0)][:, 0:bst], ht_in[256:384, 0:bst])
        nc.sync.dma_start(hT[("r", 0)][:, bst:], ht_in[0:128, bst:])
        nc.sync.dma_start(hT[("i", 0)][:, bst:], ht_in[256:384, bst:])
        nc.scalar.dma_start(w_sb[("h", 0)][:, 0:half_w],
                            w_in["h"][0:128, 0:half_w])
        nc.scalar.dma_start(hT[("r", 1)][:, 0:bst], ht_in[128:256, 0:bst])
        nc.scalar.dma_start(hT[("i", 1)][:, 0:bst], ht_in[384:512, 0:bst])
        nc.scalar.dma_start(w_sb[("h", 1)][:, 0:half_w],
                            w_in["h"][128:256, 0:half_w])
        nc.scalar.dma_start(w_sb[("h", 0)][:, half_w:],
                            w_in["h"][0:128, half_w:])
        nc.scalar.dma_start(w_sb[("h", 1)][:, half_w:],
                            w_in["h"][128:256, half_w:])
        nc.scalar.dma_start(hT[("r", 1)][:, bst:], ht_in[128:256, bst:])
        nc.scalar.dma_start(hT[("i", 1)][:, bst:], ht_in[384:512, bst:])
        if use_wlo:
            for h in (0, 1):
                nc.scalar.dma_start(
                    w_sb[("l", h)][:], w_in["l"][128 * h:128 * (h + 1), :])

        # PE warmup: dummy matmuls on zeroed SBUF while the loads
        # stream, so the PE p-state has ramped before the first real
        # matmul. No drains; the psm pool tiles are simply overwritten
        # by the real accumulation groups later.
        hz = const_pool.tile([128, 128], bf16, tag="hz", name="hz")
        wz = const_pool.tile([128, _CH], bf16, tag="wz", name="wz")
        nc.vector.memset(hz[:], 0)
        nc.vector.memset(wz[:], 0)
        for _ in range(4):
            psw = ps_mm.tile([128, 2, _CH], f32, tag="ps", name="psw")
            for xi in (0, 1):
                nc.tensor.matmul(psw[:, xi, :], hz[:], wz[:],
                                 start=True, stop=True)

        terms = [("h", "h")]
        if use_wlo:
            terms.append(("h", "l"))

        ring_of = {"s": nc.sync, "a": nc.scalar, "g": nc.gpsimd}
        copy_idx = 0
        store_idx = 0
        for bt in range(_NBT):
            # first tile(s): fine-grained stores so the write ring starts
            # as early as possible while the pipeline (and PE p-state)
            # ramps; last tile: fine-grained to shrink the tail drain
            # after the final matmul.
            se = edge_se if bt in edge_tiles else store_every
            bsl = slice(128 * bt, 128 * (bt + 1))
            ot = out_pool.tile([128, 2, _NFFT], f16, tag="ot")
            for c in range(_NCHUNK):
                pieces = pieces_per_chunk[c]
                n_mm = len(pieces) * len(terms)
                ps = ps_mm.tile([128, 2, _CH], f32, tag="ps")
                for xi, x in enumerate(("r", "i")):
                    j = 0
                    for h in pieces:
                        for (hp, wp) in terms:
                            nc.tensor.matmul(
                                ps[:, xi, :],
                                hT[(x, h)][:, bsl],
                                w_sb[(wp, h)][:, c * _CH:(c + 1) * _CH],
                                start=(j == 0),
                                stop=(j == n_mm - 1),
                            )
                            j += 1
                # one drain for the r+i pair; PSUM reads run DVE/ACT at
                # 1x, so fewer bigger casts win. Alternate engines 1:1.
                dst = ot[:, :, _CH * c:_CH * (c + 1)]
                eng = copy_cycle[copy_idx % len(copy_cycle)]
                if eng == "s":
                    nc.scalar.copy(dst, ps[:])
                else:
                    nc.vector.tensor_copy(dst, ps[:])
                copy_idx += 1
                if se >= 9:
                    # whole-tile single store after the last chunk:
                    # r and i blocks are adjacent in both SBUF and DRAM,
                    # so one DMA with 16KB/row descriptors covers both.
                    if c == _NCHUNK - 1:
                        ring = ring_of[store_rings[store_idx
                                                   % len(store_rings)]]
                        ring.dma_start(out[bass.ts(bt, 128), :],
                                       ot[:, :, :])
                        store_idx += 1
                elif (c + 1) % se == 0:
                    # store finished slices early; keeps the write ring
                    # fed and shrinks the tail drain. Alternating rings
                    # overlaps the per-ring DGE trigger latency between
                    # consecutive stores.
                    w0 = _CH * (c + 1 - se)
                    w1 = _CH * (c + 1)
                    for parity in (0, 1):
                        ring = ring_of[store_rings[store_idx
                                                   % len(store_rings)]]
                        ring.dma_start(
                            out[bass.ts(bt, 128),
                                _NFFT * parity + w0:_NFFT * parity + w1],
                            ot[:, parity, w0:w1])
                        store_idx += 1

    nc.compile()
    return nc


def _get_program(pieces, use_wlo):
    # experiment knobs (default values are the tuned ones)
    se = int(os.environ.get("K_STORE_EVERY", "9"))
    cc = os.environ.get("K_COPY_CYCLE", "vs")
    sr = os.environ.get("K_STORE_RINGS", "s")
    ese = int(os.environ.get("K_EDGE_SE", "2"))
    et = tuple(int(t) for t in
               os.environ.get("K_EDGE_TILES", "0,15").split(","))
    key = (pieces, use_wlo, se, cc, sr, ese, et)
    prog = _cache.get(key)
    if prog is None:
        prog = _build_program(pieces, use_wlo, store_every=se,
                              copy_cycle=cc, store_rings=sr,
                              edge_se=ese, edge_tiles=et)
        _cache[key] = prog
    return prog


def _make_in_maps(H_real, H_imag, W):
    w_hi, w_lo = _bf16_split(W)
    use_wlo = bool(np.any(np.asarray(w_lo) != 0))
    in_maps = []
    for i in range(_NC):
        sl = slice(i * _BS, (i + 1) * _BS)
        ht = np.ascontiguousarray(np.concatenate(
            [H_real[sl].astype(_BF16).T, H_imag[sl].astype(_BF16).T],
            axis=0))
        m = {"ht": ht, "wh": w_hi}
        if use_wlo:
            m["wl"] = w_lo
        in_maps.append(m)
    return in_maps, use_wlo


def kernel(H_real, H_imag, pilot_loc, alpha, beta):
    H_real = np.ascontiguousarray(np.asarray(H_real, dtype=np.float32))
    H_imag = np.ascontiguousarray(np.asarray(H_imag, dtype=np.float32))
    pilot_loc = np.asarray(pilot_loc, dtype=np.float32)
    alpha = np.asarray(alpha, dtype=np.float32)
    beta = np.asarray(beta, dtype=np.float32)

    W = _interp_matrix(pilot_loc, alpha, beta)
    in_maps, use_wlo = _make_in_maps(H_real, H_imag, W)
    nc = _get_program(_chunk_pieces(W), use_wlo)

    from concourse.bass_utils import run_bass_kernel_spmd

    res = run_bass_kernel_spmd(nc, in_maps, list(range(_NC))).results
    full = np.empty((_B, _NFFT, 2), dtype=np.float32)
    for i, r in enumerate(res):
        o = r["out"]
        full[i * _BS:(i + 1) * _BS, :, 0] = o[:, :_NFFT]
        full[i * _BS:(i + 1) * _BS, :, 1] = o[:, _NFFT:]
    return full
